# revision 32
# baseline (speedup 1.0000x reference)
# ChairMLP Trainium2 kernel: 8-way data-parallel over the N=65536 point batch.
#
# Per core (shard of 8192 points), computed in 16 tiles of 512 points:
#   - chain kept hidden-major [60, T]: h_k in fp32 (exact relu masks),
#     P_k streams in float32r (single-pass PE mode)
#   - outputs repacked points-major via PE transposes into [128, 960] tiles
#     (partition p holds 4 consecutive points -> 3840B contiguous DMA runs)
#   - transposes are pair-stacked: two 60-row streams at partitions [0:60]
#     and [64:124] of one tile -> one [124,128] transpose covers both
#
# Point order within tile t: column c = ch*128 + p  <->  n = R*p + 4*t + ch
# (R = nshard/128), so transposing a contiguous 128-column block ch lands
# 4 consecutive points per out-partition.

import numpy as np

import concourse.bass as bass
import concourse.tile as tile
from concourse import masks, mybir
from concourse.bass_utils import run_bass_kernel_spmd

H = 60
NCORES = 8
NSHARD = 8192
T = 512  # points per chain tile
ZB = 4   # zs epilogue batch (tiles)

F32R = mybir.dt.float32r
F32 = mybir.dt.float32
AF = mybir.ActivationFunctionType
ALU = mybir.AluOpType


def build(nshard=NSHARD, split_waits=True):
    NT = nshard // T
    R = nshard // 128
    assert R == 4 * NT and NT % ZB == 0

    nc = bass.Bass("TRN2", target_bir_lowering=False, debug=False)

    # ---------------- DRAM I/O ----------------
    x_d = nc.dram_tensor("x", [nshard, 3], F32, kind="ExternalInput")
    w1_d = nc.dram_tensor("w1", [H, 3], F32, kind="ExternalInput")
    b1_d = nc.dram_tensor("b1", [H], F32, kind="ExternalInput")
    w_d = {}
    b_d = {}
    for k in (2, 3, 4):
        w_d[k] = nc.dram_tensor(f"w{k}", [H, H], F32, kind="ExternalInput")
        b_d[k] = nc.dram_tensor(f"b{k}", [H], F32, kind="ExternalInput")
    w5_d = nc.dram_tensor("w5", [1, H], F32, kind="ExternalInput")
    b5_d = nc.dram_tensor("b5", [1], F32, kind="ExternalInput")

    out_d = nc.dram_tensor("o_out", [nshard, 1], F32, kind="ExternalOutput")
    S_d = nc.dram_tensor("o_S", [nshard, 4, H], F32, kind="ExternalOutput")
    alk_d = {
        k: nc.dram_tensor(f"o_alk{k}", [nshard, 4, H], F32, kind="ExternalOutput")
        for k in (1, 2, 3, 4)
    }
    zs_d = nc.dram_tensor("o_zs", [nshard, 4, 1], F32, kind="ExternalOutput")

    # views: n = p*R + t*4 + ch
    xv = x_d.ap().rearrange("(p r) c -> p (r c)", p=128)

    def packv(td):
        return td.ap().rearrange("(p t f) i j -> t p (f i j)", p=128, t=NT, f=4)

    alkv = {k: packv(alk_d[k]) for k in (1, 2, 3, 4)}
    Sv = packv(S_d)
    # zs/out views batched over ZB tiles: 4*ZB consecutive points / partition
    outv = out_d.ap().rearrange(
        "(p tb q) one -> tb p (q one)", p=128, tb=NT // ZB, q=4 * ZB
    )
    zsv = zs_d.ap().rearrange(
        "(p tb q) i one -> tb p (q i one)", p=128, tb=NT // ZB, q=4 * ZB
    )

    with tile.TileContext(nc) as tc:
        with (
            tc.tile_pool(name="const", bufs=1) as const,
            tc.tile_pool(name="psmm", bufs=4, space="PSUM") as psmm,
            tc.tile_pool(name="pspack", bufs=3, space="PSUM") as pspack,
            tc.tile_pool(name="xTp", bufs=2) as xTp,
            tc.tile_pool(name="hp", bufs=3) as hp,
            tc.tile_pool(name="sp", bufs=3) as sp,
            tc.tile_pool(name="Pp", bufs=2) as Pp,
            tc.tile_pool(name="ap_", bufs=4) as ap_,
            tc.tile_pool(name="aoutp", bufs=3) as aoutp,
            tc.tile_pool(name="Soutp", bufs=2) as Soutp,
            tc.tile_pool(name="zp", bufs=2) as zp,
        ):
            # ---------------- one-time setup ----------------
            ident_f = const.tile([128, 128], F32)
            masks.make_identity(nc, ident_f[:])
            ident = const.tile([128, 128], F32R)
            nc.vector.tensor_copy(ident[:], ident_f[:])

            x_sb = const.tile([128, 3 * R], F32)
            nc.sync.dma_start(x_sb[:], xv)

            # weights / biases
            w1_sb = const.tile([H, 3], F32)    # per-partition scale columns
            nc.sync.dma_start(w1_sb[:], w1_d.ap())
            b_sb = const.tile([H, 4], F32)
            nc.sync.dma_start(
                b_sb[:, 0:1], b1_d.ap().rearrange("(h one) -> h one", one=1)
            )
            for k in (2, 3, 4):
                nc.sync.dma_start(
                    b_sb[:, k - 1 : k],
                    b_d[k].ap().rearrange("(h one) -> h one", one=1),
                )
            b5_sb = const.tile([1, 1], F32)
            nc.sync.dma_start(
                b5_sb[:], b5_d.ap().rearrange("(one o2) -> one o2", o2=1)
            )

            # transposed weights: wT (f32r, P-stream matmuls) and wT_f (f32,
            # exact h-chain matmuls)
            w1f_sb = const.tile([H, 3], F32)
            nc.sync.dma_start(w1f_sb[:], w1_d.ap())
            ps_w1 = psmm.tile([3, H], F32, tag="mm")
            nc.tensor.transpose(ps_w1[:], w1f_sb[:], ident_f[0:H, 0:H])
            w1T_f = const.tile([3, H], F32)
            nc.vector.tensor_copy(w1T_f[:], ps_w1[:])
            wT_sb = {}
            wT_f = {}
            for k in (2, 3, 4):
                wk_sb = const.tile([H, H], F32, tag=f"w{k}", name=f"w{k}sb")
                nc.sync.dma_start(wk_sb[:], w_d[k].ap())
                ps_wk = psmm.tile([H, H], F32, tag="mm", name=f"pswk{k}")
                nc.tensor.transpose(ps_wk[:], wk_sb[:], ident_f[0:H, 0:H])
                wT_f[k] = const.tile([H, H], F32, tag=f"wTf{k}", name=f"wTf{k}")
                nc.vector.tensor_copy(wT_f[k][:], ps_wk[:])
                wT_sb[k] = const.tile([H, H], F32R, tag=f"wT{k}", name=f"wT{k}")
                nc.scalar.copy(wT_sb[k][:], ps_wk[:])
            w5T_f = const.tile([H, 1], F32)
            nc.sync.dma_start(
                w5T_f[:], w5_d.ap().rearrange("o (h u) -> (o h) u", u=1)
            )
            w5T_sb = const.tile([H, 1], F32R)
            nc.vector.tensor_copy(w5T_sb[:], w5T_f[:])

            # alk1 broadcast tile [128, 960], exact via doubling DMAs
            alk1_sb = const.tile([128, 960], F32)
            w1T_view = w1_d.ap().rearrange("j i -> i j")
            for i in range(3):
                nc.sync.dma_start(
                    alk1_sb[0:1, i * H : (i + 1) * H], w1T_view[i : i + 1, :]
                )
            nc.sync.dma_start(
                alk1_sb[0:1, 180:240],
                b1_d.ap().rearrange("(one h) -> one h", one=1),
            )
            for rep in range(1, 4):
                nc.vector.tensor_copy(
                    alk1_sb[0:1, rep * 240 : (rep + 1) * 240], alk1_sb[0:1, 0:240]
                )
            p = 1
            while p < 128:
                q = min(p, 128 - p)
                nc.sync.dma_start(alk1_sb[p : p + q, :], alk1_sb[0:q, :])
                p *= 2

            # ---------------- per-tile chain ----------------
            zflat = None
            for t in range(NT):
                tl = t % ZB
                # x transpose -> xT [3, T] (f32)
                ps_x = psmm.tile([3, T], F32, tag="mm")
                for ch in range(4):
                    c0 = (4 * t + ch) * 3
                    nc.tensor.transpose(
                        ps_x[0:3, ch * 128 : (ch + 1) * 128],
                        x_sb[:, c0 : c0 + 3],
                        ident_f[0:128, 0:128],
                    )
                xT = xTp.tile([3, T], F32)
                nc.scalar.copy(xT[:], ps_x[:])

                # ----- layer 1
                ps_h = psmm.tile([H, T], F32, tag="mm")
                nc.tensor.matmul(ps_h[:], w1T_f[:], xT[:])
                h = hp.tile([H, T], F32)
                nc.scalar.activation(h[:], ps_h[:], AF.Relu, bias=b_sb[:, 0:1])
                s12 = sp.tile([124, T], F32R, tag="spair", name="s12")
                nc.scalar.sign(s12[0:H, :], h[:])
                P = Pp.tile([H, 4 * T], F32R, tag="P")
                # P1_i = s1 * w1[:, i] ; B1 = s1 * b1 (ACT copy with scale)
                for i in range(3):
                    nc.scalar.activation(
                        P[:, i * T : (i + 1) * T], s12[0:H, :], AF.Copy,
                        scale=w1_sb[:, i : i + 1],
                    )
                nc.scalar.activation(
                    P[:, 3 * T : 4 * T], s12[0:H, :], AF.Copy, scale=b_sb[:, 0:1]
                )

                s34 = None
                Sout = None
                # ----- layers 2..4
                for k in (2, 3, 4):
                    ps_hk = psmm.tile([H, T], F32, tag="mm")
                    nc.tensor.matmul(ps_hk[:], wT_f[k][:], h[:])
                    ps_a = []
                    for i in range(4):
                        pa = psmm.tile([H, T], F32, tag="mm", name=f"pa{k}_{i}")
                        nc.tensor.matmul(
                            pa[:], wT_sb[k][:], P[:, i * T : (i + 1) * T]
                        )
                        ps_a.append(pa)

                    h = hp.tile([H, T], F32)
                    nc.scalar.activation(
                        h[:], ps_hk[:], AF.Relu, bias=b_sb[:, k - 1 : k]
                    )
                    if k == 2:
                        spair, soff = s12, 64
                    elif k == 3:
                        s34 = sp.tile([124, T], F32R, tag="spair", name="s34")
                        spair, soff = s34, 0
                    else:
                        spair, soff = s34, 64
                    s_sl = spair[soff : soff + H, :]
                    nc.scalar.sign(s_sl, h[:])
                    if k == 3:
                        # duplicate s3 at partition 64: the gpsimd b-stream
                        # mask needs matching base partitions with a23[64:]
                        s3b = sp.tile([124, T], F32R, tag="spair", name="s3b")
                        nc.scalar.sign(s3b[64 : 64 + H, :], h[:])
                        s_gp = s3b[64 : 64 + H, :]
                    else:
                        s_gp = s_sl

                    # alk psum -> sbuf pair tiles (for transposes)
                    a01 = ap_.tile([124, T], F32R, tag="apair", name=f"a01_{k}")
                    a23 = ap_.tile([124, T], F32R, tag="apair", name=f"a23_{k}")
                    nc.vector.tensor_copy(a01[0:H, :], ps_a[0][:])
                    nc.vector.tensor_copy(a01[64 : 64 + H, :], ps_a[1][:])
                    nc.scalar.copy(a23[0:H, :], ps_a[2][:])
                    # b-stream: +bk bias folded into the copy
                    nc.scalar.activation(
                        a23[64 : 64 + H, :], ps_a[3][:], AF.Identity,
                        bias=b_sb[:, k - 1 : k],
                    )

                    # masked P for next layer / zs
                    P = Pp.tile([H, 4 * T], F32R, tag="P")
                    for i in range(3):
                        nc.vector.tensor_tensor(
                            P[:, i * T : (i + 1) * T], ps_a[i][:], s_sl, ALU.mult
                        )
                    nc.gpsimd.tensor_tensor(
                        P[:, 3 * T : 4 * T], a23[64 : 64 + H, :], s_gp, ALU.mult
                    )

                    # pair transposes -> [128, 256]-pitch psum pack -> aout
                    aout = aoutp.tile(
                        [128, 960], F32, tag="aout", name=f"aout{k}"
                    )
                    for ch in range(4):
                        pk = pspack.tile([128, 256], F32R, tag="pack")
                        nc.tensor.transpose(
                            pk[:, 0:124],
                            a01[:, ch * 128 : (ch + 1) * 128],
                            ident[0:124, 0:124],
                        )
                        nc.tensor.transpose(
                            pk[:, 128:252],
                            a23[:, ch * 128 : (ch + 1) * 128],
                            ident[0:124, 0:124],
                        )
                        src = pk[:].rearrange(
                            "p (pr hh f) -> p pr hh f", pr=2, hh=2
                        )[:, :, :, 0:H]
                        dst = aout[:, ch * 240 : (ch + 1) * 240].rearrange(
                            "p (pr hh j) -> p pr hh j", pr=2, hh=2
                        )
                        if ch % 2 == 0:
                            nc.vector.tensor_copy(dst, src)
                        else:
                            nc.scalar.copy(dst, src)
                    nc.sync.dma_start(alkv[k][t], aout[:])

                    # S transposes: s12 during k==3, s34 during k==4
                    if k in (3, 4):
                        spr = s12 if k == 3 else s34
                        po = 0 if k == 3 else 120
                        if k == 3:
                            Sout = Soutp.tile([128, 960], F32, tag="Sout")
                        pkS = pspack.tile(
                            [128, 512], F32R, tag="pack", name="pkS"
                        )
                        for ch in range(4):
                            nc.tensor.transpose(
                                pkS[:, ch * 128 : ch * 128 + 124],
                                spr[:, ch * 128 : (ch + 1) * 128],
                                ident[0:124, 0:124],
                            )
                        for cp in range(2):
                            src = pkS[
                                :, cp * 256 : (cp + 1) * 256
                            ].rearrange(
                                "p (cc hh f) -> p cc hh f", cc=2, hh=2
                            )[:, :, :, 0:H]
                            dstb = Sout[
                                :, 2 * cp * 240 : 2 * (cp + 1) * 240
                            ].rearrange("p (cc f) -> p cc f", cc=2)[
                                :, :, po : po + 120
                            ].rearrange("p cc (hh j) -> p cc hh j", hh=2)
                            if cp == 0:
                                nc.vector.tensor_copy(dstb, src)
                            else:
                                nc.scalar.copy(dstb, src)
                    if k == 4:
                        nc.sync.dma_start(Sv[t], Sout[:])

                # alk1 broadcast write
                nc.sync.dma_start(alkv[1][t], alk1_sb[:])

                # ----- zs / out head: rows = [out, zs_w0..2, zs_b]
                if tl == 0:
                    zflat = zp.tile(
                        [1, 5 * ZB * T], F32, tag="zflat", bufs=2, name="zflat"
                    )
                for r in range(5):
                    ps_z = psmm.tile([1, T], F32, tag="mm", name=f"ps_z{r}")
                    if r == 0:
                        nc.tensor.matmul(ps_z[:], w5T_f[:], h[:])
                    else:
                        nc.tensor.matmul(
                            ps_z[:], w5T_sb[:], P[:, (r - 1) * T : r * T]
                        )
                    dstz = zflat[0:1, (r * ZB + tl) * T : (r * ZB + tl + 1) * T]
                    if r in (0, 4):
                        nc.scalar.activation(
                            dstz, ps_z[:], AF.Identity, bias=b5_sb[:]
                        )
                    else:
                        nc.vector.tensor_copy(dstz, ps_z[:])

                if tl == ZB - 1:
                    tb = t // ZB
                    z6 = zp.tile([6, ZB * T], F32, tag="z6", bufs=2, name="z6")
                    for r in range(5):
                        nc.sync.dma_start(
                            z6[r : r + 1, :],
                            zflat[0:1, r * ZB * T : (r + 1) * ZB * T],
                        )
                    nc.sync.dma_start(
                        z6[5:6, :], zflat[0:1, 4 * ZB * T : 5 * ZB * T]
                    )
                    ps_zt = pspack.tile(
                        [128, 6 * 4 * ZB], F32, tag="pack", name="ps_zt"
                    )
                    # point q = 4*tl2 + ch lives at z6[:, tl2*T + ch*128 + p]
                    for tl2 in range(ZB):
                        for ch in range(4):
                            q = 4 * tl2 + ch
                            nc.tensor.transpose(
                                ps_zt[:, q * 6 : (q + 1) * 6],
                                z6[
                                    :,
                                    tl2 * T + ch * 128 : tl2 * T + (ch + 1) * 128,
                                ],
                                ident_f[0:6, 0:6],
                            )
                    ztv = ps_zt[:].rearrange("p (q r) -> p q r", r=6)
                    out4 = zp.tile([128, 4 * ZB], F32, tag="out4", bufs=2)
                    nc.vector.tensor_copy(
                        out4[:].rearrange("p (q one) -> p q one", one=1),
                        ztv[:, :, 0:1],
                    )
                    zs16 = zp.tile([128, 16 * ZB], F32, tag="zs16", bufs=2)
                    nc.scalar.copy(
                        zs16[:].rearrange("p (q i) -> p q i", i=4),
                        ztv[:, :, 1:5],
                    )
                    nc.gpsimd.dma_start(outv[tb], out4[:])
                    nc.gpsimd.dma_start(zsv[tb], zs16[:])

    if split_waits:
        _split_excess_waits(nc)
    nc.finalize()
    return nc


def _split_excess_waits(nc):
    """walrus limits every TPB instruction to a single sync-wait command.
    Move excess waits onto injected same-engine NoOps (one wait each)."""
    n = 0
    for f in nc.m.functions:
        for b in f.blocks:
            out = []
            for inst in b.instructions:
                si = inst.sync_info
                if si is not None and si.on_wait and len(si.on_wait) > 1:
                    w = list(si.on_wait)
                    for wi in w[:-1]:
                        out.append(
                            mybir.InstNoOp(
                                name=f"wabs{n}_{inst.name}",
                                engine=inst.engine,
                                ins=[],
                                outs=[],
                                sync_info=mybir.SyncInfo(
                                    on_wait=[wi], on_update=[]
                                ),
                            )
                        )
                        n += 1
                    inst.sync_info = mybir.SyncInfo(
                        on_wait=w[-1:], on_update=list(si.on_update or [])
                    )
                out.append(inst)
            b.instructions = out
    return n


_NC_CACHE = {}


def _get_nc(nshard):
    if nshard not in _NC_CACHE:
        _NC_CACHE[nshard] = build(nshard)
    return _NC_CACHE[nshard]


def kernel(**inputs):
    x = np.ascontiguousarray(np.asarray(inputs["x"]), dtype=np.float32)
    n = x.shape[0]
    nshard = n // NCORES
    nc = _get_nc(nshard)

    weights = {
        k: np.ascontiguousarray(np.asarray(inputs[k]), dtype=np.float32)
        for k in ("w1", "b1", "w2", "b2", "w3", "b3", "w4", "b4", "w5", "b5")
    }
    in_maps = []
    for c in range(NCORES):
        m = {"x": x[c * nshard : (c + 1) * nshard]}
        m.update(weights)
        in_maps.append(m)

    res = run_bass_kernel_spmd(nc, in_maps, core_ids=list(range(NCORES)))
    outs = res.results

    def cat(name):
        return np.concatenate([outs[c][name] for c in range(NCORES)], axis=0)

    return (
        cat("o_out"),
        cat("o_S"),
        cat("o_alk1"),
        cat("o_alk2"),
        cat("o_alk3"),
        cat("o_alk4"),
        cat("o_zs"),
    )


# revision 34
# speedup vs baseline: 1.0656x; 1.0656x over previous
# ChairMLP Trainium2 kernel: 8-way data-parallel over the N=65536 point batch.
#
# Per core (shard of 8192 points), computed in 16 tiles of 512 points:
#   - chain kept hidden-major [60, T]: h_k in fp32 (exact relu masks),
#     P_k streams in float32r (single-pass PE mode)
#   - outputs repacked points-major via PE transposes into [128, 960] tiles
#     (partition p holds 4 consecutive points -> 3840B contiguous DMA runs)
#   - transposes are pair-stacked: two 60-row streams at partitions [0:60]
#     and [64:124] of one tile -> one [124,128] transpose covers both
#
# Point order within tile t: column c = ch*128 + p  <->  n = R*p + 4*t + ch
# (R = nshard/128), so transposing a contiguous 128-column block ch lands
# 4 consecutive points per out-partition.

import numpy as np

import concourse.bass as bass
import concourse.tile as tile
from concourse import masks, mybir
from concourse.bass_utils import run_bass_kernel_spmd

H = 60
NCORES = 8
NSHARD = 8192
T = 512  # points per chain tile
ZB = 4   # zs epilogue batch (tiles)

F32R = mybir.dt.float32r
F32 = mybir.dt.float32
AF = mybir.ActivationFunctionType
ALU = mybir.AluOpType


def build(nshard=NSHARD, split_waits=True):
    NT = nshard // T
    R = nshard // 128
    assert R == 4 * NT and NT % ZB == 0

    nc = bass.Bass("TRN2", target_bir_lowering=False, debug=False)

    # ---------------- DRAM I/O ----------------
    x_d = nc.dram_tensor("x", [nshard, 3], F32, kind="ExternalInput")
    w1_d = nc.dram_tensor("w1", [H, 3], F32, kind="ExternalInput")
    b1_d = nc.dram_tensor("b1", [H], F32, kind="ExternalInput")
    w_d = {}
    b_d = {}
    for k in (2, 3, 4):
        w_d[k] = nc.dram_tensor(f"w{k}", [H, H], F32, kind="ExternalInput")
        b_d[k] = nc.dram_tensor(f"b{k}", [H], F32, kind="ExternalInput")
    w5_d = nc.dram_tensor("w5", [1, H], F32, kind="ExternalInput")
    b5_d = nc.dram_tensor("b5", [1], F32, kind="ExternalInput")

    out_d = nc.dram_tensor("o_out", [nshard, 1], F32, kind="ExternalOutput")
    S_d = nc.dram_tensor("o_S", [nshard, 4, H], F32, kind="ExternalOutput")
    alk_d = {
        k: nc.dram_tensor(f"o_alk{k}", [nshard, 4, H], F32, kind="ExternalOutput")
        for k in (1, 2, 3, 4)
    }
    zs_d = nc.dram_tensor("o_zs", [nshard, 4, 1], F32, kind="ExternalOutput")

    # views: n = p*R + t*4 + ch
    xv = x_d.ap().rearrange("(p r) c -> p (r c)", p=128)

    def packv(td):
        return td.ap().rearrange("(p t f) i j -> t p (f i j)", p=128, t=NT, f=4)

    alkv = {k: packv(alk_d[k]) for k in (1, 2, 3, 4)}
    Sv = packv(S_d)
    # zs/out views batched over ZB tiles: 4*ZB consecutive points / partition
    outv = out_d.ap().rearrange(
        "(p tb q) one -> tb p (q one)", p=128, tb=NT // ZB, q=4 * ZB
    )
    zsv = zs_d.ap().rearrange(
        "(p tb q) i one -> tb p (q i one)", p=128, tb=NT // ZB, q=4 * ZB
    )

    with tile.TileContext(nc) as tc:
        with (
            tc.tile_pool(name="const", bufs=1) as const,
            tc.tile_pool(name="psmm", bufs=4, space="PSUM") as psmm,
            tc.tile_pool(name="pspack", bufs=3, space="PSUM") as pspack,
            tc.tile_pool(name="xTp", bufs=2) as xTp,
            tc.tile_pool(name="hp", bufs=3) as hp,
            tc.tile_pool(name="sp", bufs=3) as sp,
            tc.tile_pool(name="Pp", bufs=4) as Pp,
            tc.tile_pool(name="ap_", bufs=4) as ap_,
            tc.tile_pool(name="aoutp", bufs=3) as aoutp,
            tc.tile_pool(name="Soutp", bufs=2) as Soutp,
            tc.tile_pool(name="zp", bufs=2) as zp,
        ):
            # ---------------- one-time setup ----------------
            ident_f = const.tile([128, 128], F32)
            masks.make_identity(nc, ident_f[:])
            ident = const.tile([128, 128], F32R)
            nc.vector.tensor_copy(ident[:], ident_f[:])

            x_sb = const.tile([128, 3 * R], F32)
            nc.sync.dma_start(x_sb[:], xv)

            # weights / biases
            w1_sb = const.tile([H, 3], F32)    # per-partition scale columns
            nc.sync.dma_start(w1_sb[:], w1_d.ap())
            b_sb = const.tile([H, 4], F32)
            nc.sync.dma_start(
                b_sb[:, 0:1], b1_d.ap().rearrange("(h one) -> h one", one=1)
            )
            for k in (2, 3, 4):
                nc.sync.dma_start(
                    b_sb[:, k - 1 : k],
                    b_d[k].ap().rearrange("(h one) -> h one", one=1),
                )
            b5_sb = const.tile([1, 1], F32)
            nc.sync.dma_start(
                b5_sb[:], b5_d.ap().rearrange("(one o2) -> one o2", o2=1)
            )

            # transposed weights: wT (f32r, P-stream matmuls) and wT_f (f32,
            # exact h-chain matmuls)
            w1f_sb = const.tile([H, 3], F32)
            nc.sync.dma_start(w1f_sb[:], w1_d.ap())
            ps_w1 = psmm.tile([3, H], F32, tag="mm")
            nc.tensor.transpose(ps_w1[:], w1f_sb[:], ident_f[0:H, 0:H])
            w1T_f = const.tile([3, H], F32)
            nc.vector.tensor_copy(w1T_f[:], ps_w1[:])
            wT_sb = {}
            wT_f = {}
            wT64 = {}
            w2iT = {}
            for k in (2, 3, 4):
                wk_sb = const.tile([H, H], F32, tag=f"w{k}", name=f"w{k}sb")
                nc.sync.dma_start(wk_sb[:], w_d[k].ap())
                ps_wk = psmm.tile([H, H], F32, tag="mm", name=f"pswk{k}")
                nc.tensor.transpose(ps_wk[:], wk_sb[:], ident_f[0:H, 0:H])
                wT_f[k] = const.tile([H, H], F32, tag=f"wTf{k}", name=f"wTf{k}")
                nc.vector.tensor_copy(wT_f[k][:], ps_wk[:])
                wT_sb[k] = const.tile([H, H], F32R, tag=f"wT{k}", name=f"wT{k}")
                nc.scalar.copy(wT_sb[k][:], ps_wk[:])
                # copy at partitions 64:124 for row-group-paired matmuls
                wT64[k] = const.tile([124, H], F32R, tag=f"wT64{k}", name=f"wT64_{k}")
                nc.vector.tensor_copy(wT64[k][64 : 64 + H, :], ps_wk[:])
                if k == 2:
                    # P1 fold: alk2_i = (W2 diag(w1[:,i])) @ s1, so pre-scale
                    # W2^T rows by w1[:,i] (b1 for the bias stream). Streams
                    # 0,2 live at partitions 0:60; streams 1,3 at 64:124.
                    for i in range(4):
                        sc = w1_sb[:, i : i + 1] if i < 3 else b_sb[:, 0:1]
                        w2iT[i] = const.tile(
                            [124, H], F32R, tag=f"w2iT{i}", name=f"w2iT{i}"
                        )
                        off = 0 if i % 2 == 0 else 64
                        nc.scalar.activation(
                            w2iT[i][off : off + H, :], ps_wk[:], AF.Copy, scale=sc
                        )
            w5T_f = const.tile([H, 1], F32)
            nc.sync.dma_start(
                w5T_f[:], w5_d.ap().rearrange("o (h u) -> (o h) u", u=1)
            )
            w5T_sb = const.tile([H, 1], F32R)
            nc.vector.tensor_copy(w5T_sb[:], w5T_f[:])
            w5T64 = const.tile([124, 1], F32R)
            nc.vector.tensor_copy(w5T64[64 : 64 + H, :], w5T_f[:])

            # alk1 broadcast tile [128, 960], exact via doubling DMAs
            alk1_sb = const.tile([128, 960], F32)
            w1T_view = w1_d.ap().rearrange("j i -> i j")
            for i in range(3):
                nc.sync.dma_start(
                    alk1_sb[0:1, i * H : (i + 1) * H], w1T_view[i : i + 1, :]
                )
            nc.sync.dma_start(
                alk1_sb[0:1, 180:240],
                b1_d.ap().rearrange("(one h) -> one h", one=1),
            )
            for rep in range(1, 4):
                nc.vector.tensor_copy(
                    alk1_sb[0:1, rep * 240 : (rep + 1) * 240], alk1_sb[0:1, 0:240]
                )
            p = 1
            while p < 128:
                q = min(p, 128 - p)
                nc.sync.dma_start(alk1_sb[p : p + q, :], alk1_sb[0:q, :])
                p *= 2

            # ---------------- per-tile chain ----------------
            zflat = None
            for t in range(NT):
                tl = t % ZB
                # x transpose -> xT [3, T] (f32)
                ps_x = psmm.tile([3, T], F32, tag="mm")
                for ch in range(4):
                    c0 = (4 * t + ch) * 3
                    nc.tensor.transpose(
                        ps_x[0:3, ch * 128 : (ch + 1) * 128],
                        x_sb[:, c0 : c0 + 3],
                        ident_f[0:128, 0:128],
                    )
                xT = xTp.tile([3, T], F32)
                nc.scalar.copy(xT[:], ps_x[:])

                # ----- layer 1
                ps_h = psmm.tile([H, T], F32, tag="mm")
                nc.tensor.matmul(ps_h[:], w1T_f[:], xT[:])
                h = hp.tile([H, T], F32)
                nc.scalar.activation(h[:], ps_h[:], AF.Relu, bias=b_sb[:, 0:1])
                s12 = sp.tile([124, T], F32R, tag="spair", name="s12")
                nc.scalar.sign(s12[0:H, :], h[:])
                s1dup = sp.tile([124, T], F32R, tag="spair", name="s1dup")
                nc.scalar.sign(s1dup[64 : 64 + H, :], h[:])
                P01 = P23 = None

                s34 = None
                Sout = None
                # ----- layers 2..4
                for k in (2, 3, 4):
                    ps_hk = psmm.tile([H, T], F32, tag="mm")
                    nc.tensor.matmul(ps_hk[:], wT_f[k][:], h[:])
                    ps_a = []
                    for i in range(4):
                        pa = psmm.tile([H, T], F32, tag="mm", name=f"pa{k}_{i}")
                        if k == 2:
                            off = 0 if i % 2 == 0 else 64
                            rhs = s12[0:H, :] if i % 2 == 0 else s1dup[64 : 64 + H, :]
                            nc.tensor.matmul(
                                pa[:], w2iT[i][off : off + H, :], rhs
                            )
                        else:
                            src_p = P01 if i < 2 else P23
                            if i % 2 == 0:
                                nc.tensor.matmul(
                                    pa[:], wT_sb[k][:], src_p[0:H, :]
                                )
                            else:
                                nc.tensor.matmul(
                                    pa[:],
                                    wT64[k][64 : 64 + H, :],
                                    src_p[64 : 64 + H, :],
                                )
                        ps_a.append(pa)

                    h = hp.tile([H, T], F32)
                    nc.scalar.activation(
                        h[:], ps_hk[:], AF.Relu, bias=b_sb[:, k - 1 : k]
                    )
                    if k == 2:
                        spair, soff = s12, 64
                    elif k == 3:
                        s34 = sp.tile([124, T], F32R, tag="spair", name="s34")
                        spair, soff = s34, 0
                    else:
                        spair, soff = s34, 64
                    s_sl = spair[soff : soff + H, :]
                    nc.scalar.sign(s_sl, h[:])
                    if k == 3:
                        # duplicate s3 at partition 64: the gpsimd b-stream
                        # mask needs matching base partitions with a23[64:]
                        s3b = sp.tile([124, T], F32R, tag="spair", name="s3b")
                        nc.scalar.sign(s3b[64 : 64 + H, :], h[:])
                        s_gp = s3b[64 : 64 + H, :]
                    else:
                        s_gp = s_sl

                    # alk psum -> sbuf pair tiles (for transposes)
                    a01 = ap_.tile([124, T], F32R, tag="apair", name=f"a01_{k}")
                    a23 = ap_.tile([124, T], F32R, tag="apair", name=f"a23_{k}")
                    nc.vector.tensor_copy(a01[0:H, :], ps_a[0][:])
                    nc.vector.tensor_copy(a01[64 : 64 + H, :], ps_a[1][:])
                    nc.scalar.copy(a23[0:H, :], ps_a[2][:])
                    # b-stream: +bk bias folded into the copy
                    nc.scalar.activation(
                        a23[64 : 64 + H, :], ps_a[3][:], AF.Identity,
                        bias=b_sb[:, k - 1 : k],
                    )

                    # masked P (pair-stacked for row-group-paired matmuls)
                    P01 = Pp.tile([124, T], F32R, tag="P", name=f"P01_{k}")
                    P23 = Pp.tile([124, T], F32R, tag="P", name=f"P23_{k}")
                    nc.vector.tensor_tensor(P01[0:H, :], ps_a[0][:], s_sl, ALU.mult)
                    nc.vector.tensor_tensor(
                        P01[64 : 64 + H, :], ps_a[1][:], s_sl, ALU.mult
                    )
                    nc.vector.tensor_tensor(P23[0:H, :], ps_a[2][:], s_sl, ALU.mult)
                    nc.gpsimd.tensor_tensor(
                        P23[64 : 64 + H, :], a23[64 : 64 + H, :], s_gp, ALU.mult
                    )

                    # pair transposes -> [128, 256]-pitch psum pack -> aout
                    aout = aoutp.tile(
                        [128, 960], F32, tag="aout", name=f"aout{k}"
                    )
                    for ch in range(4):
                        pk = pspack.tile([128, 256], F32R, tag="pack")
                        nc.tensor.transpose(
                            pk[:, 0:124],
                            a01[:, ch * 128 : (ch + 1) * 128],
                            ident[0:124, 0:124],
                        )
                        nc.tensor.transpose(
                            pk[:, 128:252],
                            a23[:, ch * 128 : (ch + 1) * 128],
                            ident[0:124, 0:124],
                        )
                        src = pk[:].rearrange(
                            "p (pr hh f) -> p pr hh f", pr=2, hh=2
                        )[:, :, :, 0:H]
                        dst = aout[:, ch * 240 : (ch + 1) * 240].rearrange(
                            "p (pr hh j) -> p pr hh j", pr=2, hh=2
                        )
                        if ch % 2 == 0:
                            nc.vector.tensor_copy(dst, src)
                        else:
                            nc.scalar.copy(dst, src)
                    nc.sync.dma_start(alkv[k][t], aout[:])

                    # S transposes: s12 during k==3, s34 during k==4
                    if k in (3, 4):
                        spr = s12 if k == 3 else s34
                        po = 0 if k == 3 else 120
                        if k == 3:
                            Sout = Soutp.tile([128, 960], F32, tag="Sout")
                        pkS = pspack.tile(
                            [128, 512], F32R, tag="pack", name="pkS"
                        )
                        for ch in range(4):
                            nc.tensor.transpose(
                                pkS[:, ch * 128 : ch * 128 + 124],
                                spr[:, ch * 128 : (ch + 1) * 128],
                                ident[0:124, 0:124],
                            )
                        for cp in range(2):
                            src = pkS[
                                :, cp * 256 : (cp + 1) * 256
                            ].rearrange(
                                "p (cc hh f) -> p cc hh f", cc=2, hh=2
                            )[:, :, :, 0:H]
                            dstb = Sout[
                                :, 2 * cp * 240 : 2 * (cp + 1) * 240
                            ].rearrange("p (cc f) -> p cc f", cc=2)[
                                :, :, po : po + 120
                            ].rearrange("p cc (hh j) -> p cc hh j", hh=2)
                            if cp == 0:
                                nc.vector.tensor_copy(dstb, src)
                            else:
                                nc.scalar.copy(dstb, src)
                    if k == 4:
                        nc.sync.dma_start(Sv[t], Sout[:])

                # alk1 broadcast write
                nc.sync.dma_start(alkv[1][t], alk1_sb[:])

                # ----- zs / out head: rows = [out, zs_w0..2, zs_b]
                if tl == 0:
                    zflat = zp.tile(
                        [1, 5 * ZB * T], F32, tag="zflat", bufs=2, name="zflat"
                    )
                for r in range(5):
                    ps_z = psmm.tile([1, T], F32, tag="mm", name=f"ps_z{r}")
                    if r == 0:
                        nc.tensor.matmul(ps_z[:], w5T_f[:], h[:])
                    elif r in (1, 3):
                        srcp = P01 if r == 1 else P23
                        nc.tensor.matmul(ps_z[:], w5T_sb[:], srcp[0:H, :])
                    else:
                        srcp = P01 if r == 2 else P23
                        nc.tensor.matmul(
                            ps_z[:], w5T64[64 : 64 + H, :], srcp[64 : 64 + H, :]
                        )
                    dstz = zflat[0:1, (r * ZB + tl) * T : (r * ZB + tl + 1) * T]
                    if r in (0, 4):
                        nc.scalar.activation(
                            dstz, ps_z[:], AF.Identity, bias=b5_sb[:]
                        )
                    else:
                        nc.vector.tensor_copy(dstz, ps_z[:])

                if tl == ZB - 1:
                    tb = t // ZB
                    z6 = zp.tile([6, ZB * T], F32, tag="z6", bufs=2, name="z6")
                    for r in range(5):
                        nc.sync.dma_start(
                            z6[r : r + 1, :],
                            zflat[0:1, r * ZB * T : (r + 1) * ZB * T],
                        )
                    nc.sync.dma_start(
                        z6[5:6, :], zflat[0:1, 4 * ZB * T : 5 * ZB * T]
                    )
                    ps_zt = pspack.tile(
                        [128, 6 * 4 * ZB], F32, tag="pack", name="ps_zt"
                    )
                    # point q = 4*tl2 + ch lives at z6[:, tl2*T + ch*128 + p]
                    for tl2 in range(ZB):
                        for ch in range(4):
                            q = 4 * tl2 + ch
                            nc.tensor.transpose(
                                ps_zt[:, q * 6 : (q + 1) * 6],
                                z6[
                                    :,
                                    tl2 * T + ch * 128 : tl2 * T + (ch + 1) * 128,
                                ],
                                ident_f[0:6, 0:6],
                            )
                    ztv = ps_zt[:].rearrange("p (q r) -> p q r", r=6)
                    out4 = zp.tile([128, 4 * ZB], F32, tag="out4", bufs=2)
                    nc.vector.tensor_copy(
                        out4[:].rearrange("p (q one) -> p q one", one=1),
                        ztv[:, :, 0:1],
                    )
                    zs16 = zp.tile([128, 16 * ZB], F32, tag="zs16", bufs=2)
                    nc.scalar.copy(
                        zs16[:].rearrange("p (q i) -> p q i", i=4),
                        ztv[:, :, 1:5],
                    )
                    nc.gpsimd.dma_start(outv[tb], out4[:])
                    nc.gpsimd.dma_start(zsv[tb], zs16[:])

    if split_waits:
        _split_excess_waits(nc)
    nc.finalize()
    return nc


def _split_excess_waits(nc):
    """walrus limits every TPB instruction to a single sync-wait command.
    Move excess waits onto injected same-engine NoOps (one wait each)."""
    n = 0
    for f in nc.m.functions:
        for b in f.blocks:
            out = []
            for inst in b.instructions:
                si = inst.sync_info
                if si is not None and si.on_wait and len(si.on_wait) > 1:
                    w = list(si.on_wait)
                    for wi in w[:-1]:
                        out.append(
                            mybir.InstNoOp(
                                name=f"wabs{n}_{inst.name}",
                                engine=inst.engine,
                                ins=[],
                                outs=[],
                                sync_info=mybir.SyncInfo(
                                    on_wait=[wi], on_update=[]
                                ),
                            )
                        )
                        n += 1
                    inst.sync_info = mybir.SyncInfo(
                        on_wait=w[-1:], on_update=list(si.on_update or [])
                    )
                out.append(inst)
            b.instructions = out
    return n


_NC_CACHE = {}


def _get_nc(nshard):
    if nshard not in _NC_CACHE:
        _NC_CACHE[nshard] = build(nshard)
    return _NC_CACHE[nshard]


def kernel(**inputs):
    x = np.ascontiguousarray(np.asarray(inputs["x"]), dtype=np.float32)
    n = x.shape[0]
    nshard = n // NCORES
    nc = _get_nc(nshard)

    weights = {
        k: np.ascontiguousarray(np.asarray(inputs[k]), dtype=np.float32)
        for k in ("w1", "b1", "w2", "b2", "w3", "b3", "w4", "b4", "w5", "b5")
    }
    in_maps = []
    for c in range(NCORES):
        m = {"x": x[c * nshard : (c + 1) * nshard]}
        m.update(weights)
        in_maps.append(m)

    res = run_bass_kernel_spmd(nc, in_maps, core_ids=list(range(NCORES)))
    outs = res.results

    def cat(name):
        return np.concatenate([outs[c][name] for c in range(NCORES)], axis=0)

    return (
        cat("o_out"),
        cat("o_S"),
        cat("o_alk1"),
        cat("o_alk2"),
        cat("o_alk3"),
        cat("o_alk4"),
        cat("o_zs"),
    )


# revision 35
# speedup vs baseline: 1.2191x; 1.1440x over previous
# ChairMLP Trainium2 kernel: 8-way data-parallel over the N=65536 point batch.
#
# Per core (shard of 8192 points), computed in 16 tiles of 512 points:
#   - chain kept hidden-major [60, T]: h_k in fp32 (exact relu masks),
#     P_k streams in float32r (single-pass PE mode)
#   - outputs repacked points-major via PE transposes into [128, 960] tiles
#     (partition p holds 4 consecutive points -> 3840B contiguous DMA runs)
#   - transposes are pair-stacked: two 60-row streams at partitions [0:60]
#     and [64:124] of one tile -> one [124,128] transpose covers both
#
# Point order within tile t: column c = ch*128 + p  <->  n = R*p + 4*t + ch
# (R = nshard/128), so transposing a contiguous 128-column block ch lands
# 4 consecutive points per out-partition.

import numpy as np

import concourse.bass as bass
import concourse.tile as tile
from concourse import masks, mybir
from concourse.bass_utils import run_bass_kernel_spmd

H = 60
NCORES = 8
NSHARD = 8192
T = 512  # points per chain tile
ZB = 2   # zs epilogue batch (tiles)

F32R = mybir.dt.float32r
F32 = mybir.dt.float32
AF = mybir.ActivationFunctionType
ALU = mybir.AluOpType


def build(nshard=NSHARD, split_waits=True):
    NT = nshard // T
    R = nshard // 128
    assert R == 4 * NT and NT % ZB == 0

    nc = bass.Bass("TRN2", target_bir_lowering=False, debug=False)

    # ---------------- DRAM I/O ----------------
    x_d = nc.dram_tensor("x", [nshard, 3], F32, kind="ExternalInput")
    w1_d = nc.dram_tensor("w1", [H, 3], F32, kind="ExternalInput")
    b1_d = nc.dram_tensor("b1", [H], F32, kind="ExternalInput")
    w_d = {}
    b_d = {}
    for k in (2, 3, 4):
        w_d[k] = nc.dram_tensor(f"w{k}", [H, H], F32, kind="ExternalInput")
        b_d[k] = nc.dram_tensor(f"b{k}", [H], F32, kind="ExternalInput")
    w5_d = nc.dram_tensor("w5", [1, H], F32, kind="ExternalInput")
    b5_d = nc.dram_tensor("b5", [1], F32, kind="ExternalInput")

    out_d = nc.dram_tensor("o_out", [nshard, 1], F32, kind="ExternalOutput")
    S_d = nc.dram_tensor("o_S", [nshard, 4, H], F32, kind="ExternalOutput")
    alk_d = {
        k: nc.dram_tensor(f"o_alk{k}", [nshard, 4, H], F32, kind="ExternalOutput")
        for k in (1, 2, 3, 4)
    }
    zs_d = nc.dram_tensor("o_zs", [nshard, 4, 1], F32, kind="ExternalOutput")

    # views: n = p*R + t*4 + ch
    xv = x_d.ap().rearrange("(p r) c -> p (r c)", p=128)

    def packv(td):
        return td.ap().rearrange("(p t f) i j -> t p (f i j)", p=128, t=NT, f=4)

    alkv = {k: packv(alk_d[k]) for k in (1, 2, 3, 4)}
    Sv = packv(S_d)
    # zs/out views batched over ZB tiles: 4*ZB consecutive points / partition
    outv = out_d.ap().rearrange(
        "(p tb q) one -> tb p (q one)", p=128, tb=NT // ZB, q=4 * ZB
    )
    zsv = zs_d.ap().rearrange(
        "(p tb q) i one -> tb p (q i one)", p=128, tb=NT // ZB, q=4 * ZB
    )

    with tile.TileContext(nc) as tc:
        with (
            tc.tile_pool(name="const", bufs=1) as const,
            tc.tile_pool(name="psmm", bufs=5, space="PSUM") as psmm,
            tc.tile_pool(name="pspack", bufs=3, space="PSUM") as pspack,
            tc.tile_pool(name="xTp", bufs=3) as xTp,
            tc.tile_pool(name="hp", bufs=5) as hp,
            tc.tile_pool(name="sp", bufs=8) as sp,
            tc.tile_pool(name="Pp", bufs=6) as Pp,
            tc.tile_pool(name="ap_", bufs=6) as ap_,
            tc.tile_pool(name="aoutp", bufs=4) as aoutp,
            tc.tile_pool(name="Soutp", bufs=3) as Soutp,
            tc.tile_pool(name="zp", bufs=2) as zp,
        ):
            # ---------------- one-time setup ----------------
            ident_f = const.tile([128, 128], F32)
            masks.make_identity(nc, ident_f[:])
            ident = const.tile([128, 128], F32R)
            nc.vector.tensor_copy(ident[:], ident_f[:])

            x_sb = const.tile([128, 3 * R], F32)
            nc.sync.dma_start(x_sb[:], xv)

            # weights / biases
            w1_sb = const.tile([H, 3], F32)    # per-partition scale columns
            nc.sync.dma_start(w1_sb[:], w1_d.ap())
            b_sb = const.tile([H, 4], F32)
            nc.sync.dma_start(
                b_sb[:, 0:1], b1_d.ap().rearrange("(h one) -> h one", one=1)
            )
            for k in (2, 3, 4):
                nc.sync.dma_start(
                    b_sb[:, k - 1 : k],
                    b_d[k].ap().rearrange("(h one) -> h one", one=1),
                )
            b5_sb = const.tile([1, 1], F32)
            nc.sync.dma_start(
                b5_sb[:], b5_d.ap().rearrange("(one o2) -> one o2", o2=1)
            )

            # transposed weights: wT (f32r, P-stream matmuls) and wT_f (f32,
            # exact h-chain matmuls)
            w1f_sb = const.tile([H, 3], F32)
            nc.sync.dma_start(w1f_sb[:], w1_d.ap())
            ps_w1 = psmm.tile([3, H], F32, tag="mm")
            nc.tensor.transpose(ps_w1[:], w1f_sb[:], ident_f[0:H, 0:H])
            w1T_f = const.tile([3, H], F32)
            nc.vector.tensor_copy(w1T_f[:], ps_w1[:])
            wT_sb = {}
            wT_f = {}
            wT64 = {}
            w2iT = {}
            for k in (2, 3, 4):
                wk_sb = const.tile([H, H], F32, tag=f"w{k}", name=f"w{k}sb")
                nc.sync.dma_start(wk_sb[:], w_d[k].ap())
                ps_wk = psmm.tile([H, H], F32, tag="mm", name=f"pswk{k}")
                nc.tensor.transpose(ps_wk[:], wk_sb[:], ident_f[0:H, 0:H])
                wT_f[k] = const.tile([H, H], F32, tag=f"wTf{k}", name=f"wTf{k}")
                nc.vector.tensor_copy(wT_f[k][:], ps_wk[:])
                wT_sb[k] = const.tile([H, H], F32R, tag=f"wT{k}", name=f"wT{k}")
                nc.scalar.copy(wT_sb[k][:], ps_wk[:])
                # copy at partitions 64:124 for row-group-paired matmuls
                wT64[k] = const.tile([124, H], F32R, tag=f"wT64{k}", name=f"wT64_{k}")
                nc.vector.tensor_copy(wT64[k][64 : 64 + H, :], ps_wk[:])
                if k == 2:
                    # P1 fold: alk2_i = (W2 diag(w1[:,i])) @ s1, so pre-scale
                    # W2^T rows by w1[:,i] (b1 for the bias stream). Streams
                    # 0,2 live at partitions 0:60; streams 1,3 at 64:124.
                    for i in range(4):
                        sc = w1_sb[:, i : i + 1] if i < 3 else b_sb[:, 0:1]
                        w2iT[i] = const.tile(
                            [124, H], F32R, tag=f"w2iT{i}", name=f"w2iT{i}"
                        )
                        off = 0 if i % 2 == 0 else 64
                        nc.scalar.activation(
                            w2iT[i][off : off + H, :], ps_wk[:], AF.Copy, scale=sc
                        )
            w5T_f = const.tile([H, 1], F32)
            nc.sync.dma_start(
                w5T_f[:], w5_d.ap().rearrange("o (h u) -> (o h) u", u=1)
            )
            w5T_sb = const.tile([H, 1], F32R)
            nc.vector.tensor_copy(w5T_sb[:], w5T_f[:])
            w5T64 = const.tile([124, 1], F32R)
            nc.vector.tensor_copy(w5T64[64 : 64 + H, :], w5T_f[:])

            # alk1 broadcast tile [128, 960], exact via doubling DMAs
            alk1_sb = const.tile([128, 960], F32)
            w1T_view = w1_d.ap().rearrange("j i -> i j")
            for i in range(3):
                nc.sync.dma_start(
                    alk1_sb[0:1, i * H : (i + 1) * H], w1T_view[i : i + 1, :]
                )
            nc.sync.dma_start(
                alk1_sb[0:1, 180:240],
                b1_d.ap().rearrange("(one h) -> one h", one=1),
            )
            for rep in range(1, 4):
                nc.vector.tensor_copy(
                    alk1_sb[0:1, rep * 240 : (rep + 1) * 240], alk1_sb[0:1, 0:240]
                )
            p = 1
            while p < 128:
                q = min(p, 128 - p)
                nc.sync.dma_start(alk1_sb[p : p + q, :], alk1_sb[0:q, :])
                p *= 2

            # ---------------- per-tile chain ----------------
            zflat = None
            for t in range(NT):
                tl = t % ZB
                # x transpose -> xT [3, T] (f32)
                ps_x = psmm.tile([3, T], F32, tag="mm")
                for ch in range(4):
                    c0 = (4 * t + ch) * 3
                    nc.tensor.transpose(
                        ps_x[0:3, ch * 128 : (ch + 1) * 128],
                        x_sb[:, c0 : c0 + 3],
                        ident_f[0:128, 0:128],
                    )
                xT = xTp.tile([3, T], F32)
                nc.scalar.copy(xT[:], ps_x[:])

                # ----- layer 1
                ps_h = psmm.tile([H, T], F32, tag="mm")
                nc.tensor.matmul(ps_h[:], w1T_f[:], xT[:])
                h = hp.tile([H, T], F32)
                nc.scalar.activation(h[:], ps_h[:], AF.Relu, bias=b_sb[:, 0:1])
                s12 = sp.tile([124, T], F32R, tag="spair", name="s12")
                nc.scalar.sign(s12[0:H, :], h[:])
                s1dup = sp.tile([124, T], F32R, tag="spair", name="s1dup")
                nc.scalar.sign(s1dup[64 : 64 + H, :], h[:])
                P01 = P23 = None

                s34 = None
                Sout = None
                # ----- layers 2..4
                for k in (2, 3, 4):
                    ps_hk = psmm.tile([H, T], F32, tag="mm")
                    nc.tensor.matmul(ps_hk[:], wT_f[k][:], h[:])
                    ps_a = []
                    for i in range(4):
                        pa = psmm.tile([H, T], F32, tag="mm", name=f"pa{k}_{i}")
                        if k == 2:
                            off = 0 if i % 2 == 0 else 64
                            rhs = s12[0:H, :] if i % 2 == 0 else s1dup[64 : 64 + H, :]
                            nc.tensor.matmul(
                                pa[:], w2iT[i][off : off + H, :], rhs
                            )
                        else:
                            src_p = P01 if i < 2 else P23
                            if i % 2 == 0:
                                nc.tensor.matmul(
                                    pa[:], wT_sb[k][:], src_p[0:H, :]
                                )
                            else:
                                nc.tensor.matmul(
                                    pa[:],
                                    wT64[k][64 : 64 + H, :],
                                    src_p[64 : 64 + H, :],
                                )
                        ps_a.append(pa)

                    h = hp.tile([H, T], F32)
                    nc.scalar.activation(
                        h[:], ps_hk[:], AF.Relu, bias=b_sb[:, k - 1 : k]
                    )
                    if k == 2:
                        spair, soff = s12, 64
                    elif k == 3:
                        s34 = sp.tile([124, T], F32R, tag="spair", name="s34")
                        spair, soff = s34, 0
                    else:
                        spair, soff = s34, 64
                    s_sl = spair[soff : soff + H, :]
                    nc.scalar.sign(s_sl, h[:])
                    if k == 3:
                        # duplicate s3 at partition 64: the gpsimd b-stream
                        # mask needs matching base partitions with a23[64:]
                        s3b = sp.tile([124, T], F32R, tag="spair", name="s3b")
                        nc.scalar.sign(s3b[64 : 64 + H, :], h[:])
                        s_gp = s3b[64 : 64 + H, :]
                    else:
                        s_gp = s_sl

                    # alk psum -> sbuf pair tiles (for transposes)
                    a01 = ap_.tile([124, T], F32R, tag="apair", name=f"a01_{k}")
                    a23 = ap_.tile([124, T], F32R, tag="apair", name=f"a23_{k}")
                    nc.vector.tensor_copy(a01[0:H, :], ps_a[0][:])
                    nc.vector.tensor_copy(a01[64 : 64 + H, :], ps_a[1][:])
                    nc.scalar.copy(a23[0:H, :], ps_a[2][:])
                    # b-stream: +bk bias folded into the copy
                    nc.scalar.activation(
                        a23[64 : 64 + H, :], ps_a[3][:], AF.Identity,
                        bias=b_sb[:, k - 1 : k],
                    )

                    # masked P (pair-stacked for row-group-paired matmuls)
                    P01 = Pp.tile([124, T], F32R, tag="P", name=f"P01_{k}")
                    P23 = Pp.tile([124, T], F32R, tag="P", name=f"P23_{k}")
                    nc.vector.tensor_tensor(P01[0:H, :], ps_a[0][:], s_sl, ALU.mult)
                    nc.vector.tensor_tensor(
                        P01[64 : 64 + H, :], ps_a[1][:], s_sl, ALU.mult
                    )
                    nc.vector.tensor_tensor(P23[0:H, :], ps_a[2][:], s_sl, ALU.mult)
                    nc.gpsimd.tensor_tensor(
                        P23[64 : 64 + H, :], a23[64 : 64 + H, :], s_gp, ALU.mult
                    )

                    # pair transposes -> [128, 256]-pitch psum pack -> aout
                    aout = aoutp.tile(
                        [128, 960], F32, tag="aout", name=f"aout{k}"
                    )
                    for ch in range(4):
                        pk = pspack.tile([128, 256], F32R, tag="pack")
                        nc.tensor.transpose(
                            pk[:, 0:124],
                            a01[:, ch * 128 : (ch + 1) * 128],
                            ident[0:124, 0:124],
                        )
                        nc.tensor.transpose(
                            pk[:, 128:252],
                            a23[:, ch * 128 : (ch + 1) * 128],
                            ident[0:124, 0:124],
                        )
                        src = pk[:].rearrange(
                            "p (pr hh f) -> p pr hh f", pr=2, hh=2
                        )[:, :, :, 0:H]
                        dst = aout[:, ch * 240 : (ch + 1) * 240].rearrange(
                            "p (pr hh j) -> p pr hh j", pr=2, hh=2
                        )
                        if ch % 2 == 0:
                            nc.vector.tensor_copy(dst, src)
                        else:
                            nc.scalar.copy(dst, src)
                    nc.sync.dma_start(alkv[k][t], aout[:])

                    # S transposes: s12 during k==3, s34 during k==4
                    if k in (3, 4):
                        spr = s12 if k == 3 else s34
                        po = 0 if k == 3 else 120
                        if k == 3:
                            Sout = Soutp.tile([128, 960], F32, tag="Sout")
                        pkS = pspack.tile(
                            [128, 512], F32R, tag="pack", name="pkS"
                        )
                        for ch in range(4):
                            nc.tensor.transpose(
                                pkS[:, ch * 128 : ch * 128 + 124],
                                spr[:, ch * 128 : (ch + 1) * 128],
                                ident[0:124, 0:124],
                            )
                        for cp in range(2):
                            src = pkS[
                                :, cp * 256 : (cp + 1) * 256
                            ].rearrange(
                                "p (cc hh f) -> p cc hh f", cc=2, hh=2
                            )[:, :, :, 0:H]
                            dstb = Sout[
                                :, 2 * cp * 240 : 2 * (cp + 1) * 240
                            ].rearrange("p (cc f) -> p cc f", cc=2)[
                                :, :, po : po + 120
                            ].rearrange("p cc (hh j) -> p cc hh j", hh=2)
                            if cp == 0:
                                nc.vector.tensor_copy(dstb, src)
                            else:
                                nc.scalar.copy(dstb, src)
                    if k == 4:
                        nc.sync.dma_start(Sv[t], Sout[:])

                # alk1 broadcast write
                nc.sync.dma_start(alkv[1][t], alk1_sb[:])

                # ----- zs / out head: rows = [out, zs_w0..2, zs_b]
                if tl == 0:
                    zflat = zp.tile(
                        [1, 5 * ZB * T], F32, tag="zflat", bufs=2, name="zflat"
                    )
                for r in range(5):
                    ps_z = psmm.tile([1, T], F32, tag="mm", name=f"ps_z{r}")
                    if r == 0:
                        nc.tensor.matmul(ps_z[:], w5T_f[:], h[:])
                    elif r in (1, 3):
                        srcp = P01 if r == 1 else P23
                        nc.tensor.matmul(ps_z[:], w5T_sb[:], srcp[0:H, :])
                    else:
                        srcp = P01 if r == 2 else P23
                        nc.tensor.matmul(
                            ps_z[:], w5T64[64 : 64 + H, :], srcp[64 : 64 + H, :]
                        )
                    dstz = zflat[0:1, (r * ZB + tl) * T : (r * ZB + tl + 1) * T]
                    if r in (0, 4):
                        nc.scalar.activation(
                            dstz, ps_z[:], AF.Identity, bias=b5_sb[:]
                        )
                    else:
                        nc.vector.tensor_copy(dstz, ps_z[:])

                if tl == ZB - 1:
                    tb = t // ZB
                    z6 = zp.tile([6, ZB * T], F32, tag="z6", bufs=2, name="z6")
                    for r in range(5):
                        nc.sync.dma_start(
                            z6[r : r + 1, :],
                            zflat[0:1, r * ZB * T : (r + 1) * ZB * T],
                        )
                    nc.sync.dma_start(
                        z6[5:6, :], zflat[0:1, 4 * ZB * T : 5 * ZB * T]
                    )
                    ps_zt = pspack.tile(
                        [128, 6 * 4 * ZB], F32, tag="pack", name="ps_zt"
                    )
                    # point q = 4*tl2 + ch lives at z6[:, tl2*T + ch*128 + p]
                    for tl2 in range(ZB):
                        for ch in range(4):
                            q = 4 * tl2 + ch
                            nc.tensor.transpose(
                                ps_zt[:, q * 6 : (q + 1) * 6],
                                z6[
                                    :,
                                    tl2 * T + ch * 128 : tl2 * T + (ch + 1) * 128,
                                ],
                                ident_f[0:6, 0:6],
                            )
                    ztv = ps_zt[:].rearrange("p (q r) -> p q r", r=6)
                    out4 = zp.tile([128, 4 * ZB], F32, tag="out4", bufs=2)
                    nc.vector.tensor_copy(
                        out4[:].rearrange("p (q one) -> p q one", one=1),
                        ztv[:, :, 0:1],
                    )
                    zs16 = zp.tile([128, 16 * ZB], F32, tag="zs16", bufs=2)
                    nc.scalar.copy(
                        zs16[:].rearrange("p (q i) -> p q i", i=4),
                        ztv[:, :, 1:5],
                    )
                    nc.gpsimd.dma_start(outv[tb], out4[:])
                    nc.gpsimd.dma_start(zsv[tb], zs16[:])

    if split_waits:
        _split_excess_waits(nc)
    nc.finalize()
    return nc


def _split_excess_waits(nc):
    """walrus limits every TPB instruction to a single sync-wait command.
    Move excess waits onto injected same-engine NoOps (one wait each)."""
    n = 0
    for f in nc.m.functions:
        for b in f.blocks:
            out = []
            for inst in b.instructions:
                si = inst.sync_info
                if si is not None and si.on_wait and len(si.on_wait) > 1:
                    w = list(si.on_wait)
                    for wi in w[:-1]:
                        out.append(
                            mybir.InstNoOp(
                                name=f"wabs{n}_{inst.name}",
                                engine=inst.engine,
                                ins=[],
                                outs=[],
                                sync_info=mybir.SyncInfo(
                                    on_wait=[wi], on_update=[]
                                ),
                            )
                        )
                        n += 1
                    inst.sync_info = mybir.SyncInfo(
                        on_wait=w[-1:], on_update=list(si.on_update or [])
                    )
                out.append(inst)
            b.instructions = out
    return n


_NC_CACHE = {}


def _get_nc(nshard):
    if nshard not in _NC_CACHE:
        _NC_CACHE[nshard] = build(nshard)
    return _NC_CACHE[nshard]


def kernel(**inputs):
    x = np.ascontiguousarray(np.asarray(inputs["x"]), dtype=np.float32)
    n = x.shape[0]
    nshard = n // NCORES
    nc = _get_nc(nshard)

    weights = {
        k: np.ascontiguousarray(np.asarray(inputs[k]), dtype=np.float32)
        for k in ("w1", "b1", "w2", "b2", "w3", "b3", "w4", "b4", "w5", "b5")
    }
    in_maps = []
    for c in range(NCORES):
        m = {"x": x[c * nshard : (c + 1) * nshard]}
        m.update(weights)
        in_maps.append(m)

    res = run_bass_kernel_spmd(nc, in_maps, core_ids=list(range(NCORES)))
    outs = res.results

    def cat(name):
        return np.concatenate([outs[c][name] for c in range(NCORES)], axis=0)

    return (
        cat("o_out"),
        cat("o_S"),
        cat("o_alk1"),
        cat("o_alk2"),
        cat("o_alk3"),
        cat("o_alk4"),
        cat("o_zs"),
    )
